# revision 2
# baseline (speedup 1.0000x reference)
"""Trainium2 Bass kernel for a 2-layer GCN (AttributeDecoder):

    out = relu(adj @ relu(adj @ (X @ W1)) @ W2)

with N=8192, D_IN=64, D_HID=128, D_OUT=256, all fp32.

Strategy (8 NeuronCores, SPMD):
  - Row-shard adj across cores: core i owns rows [1024*i, 1024*(i+1)).
    The host feeds each core adjT_i = adj[rows_i, :].T  ([8192, 1024],
    contiguous) so that on-chip tiles naturally have the contraction
    index k on the partition axis (the PE reduces over partitions).
  - X is passed transposed (xT [64, 8192]) and replicated; W1/W2 replicated.
  - On-chip per core:
      XW1 = X @ W1                     (f32r matmuls, [8192, 128] in SBUF)
      H1^T_own = relu(adj_i @ XW1)^T   psum[n,m] += XW1[kblk].T @ adjT slab
      G_own = H1_own @ W2              (exact fp32 matmuls, -> f32r in DRAM)
      AllGather(G_own) -> G [8192, 256] (f32r)
      OUT^T_own = relu(adj_i @ G)^T    psum[c,m] += G[kblk].T @ adjT slab
  - Host gathers outT_i ([256, 1024]) from each core and transposes back.

All adj/XW1/G matmuls use float32r (TF32-like, ~1.6e-4 rel err, full PE
rate at moving dim >= 256). Raw fp32 bits are DMA'd into f32r-declared
tensors directly; the hardware rounds internally - no cast pass needed.
The first NCACHE adjT slabs stay resident in SBUF after layer 1 so layer 2
skips their HBM reads (target_regime=memory).
"""

import numpy as np

N = 8192
D_IN, D_HID, D_OUT = 64, 128, 256
NCORES = 8
SHARD = N // NCORES  # 1024
KB = N // 128  # 64 k-blocks of 128
KB_OWN = SHARD // 128  # 8 k-blocks owned per core
# Number of k-block slabs ([128, SHARD] f32r = 512KB each) kept resident in
# SBUF after layer 1 so layer 2 skips their HBM reads.
NCACHE = 24


def _build_nc(reps: int = 1):
    from concourse import bacc
    import concourse.mybir as mybir
    import concourse.tile as tile
    from concourse.bass import ts

    f32 = mybir.dt.float32
    f32r = mybir.dt.float32r
    Relu = mybir.ActivationFunctionType.Relu

    nc = bacc.Bacc("TRN2", target_bir_lowering=False, debug=False, num_devices=NCORES)

    adjT = nc.dram_tensor("adjT", [N, SHARD], f32r, kind="ExternalInput").ap()
    xT = nc.dram_tensor("xT", [D_IN, N], f32r, kind="ExternalInput").ap()
    w1 = nc.dram_tensor("w1", [D_IN, D_HID], f32r, kind="ExternalInput").ap()
    w2 = nc.dram_tensor("w2", [D_HID, D_OUT], f32, kind="ExternalInput").ap()
    outT = nc.dram_tensor("outT", [D_OUT, SHARD], f32, kind="ExternalOutput").ap()

    def body(tc, rep):
        nc = tc.nc
        with (
            tc.tile_pool(name="const", bufs=1) as const_pool,
            tc.tile_pool(name="cache", bufs=1) as cache_pool,
            tc.tile_pool(name="slab", bufs=4) as slab_pool,
            tc.tile_pool(name="gt", bufs=3) as gt_pool,
            tc.tile_pool(name="copies", bufs=2) as copy_pool,
            tc.tile_pool(name="dram", bufs=1, space="DRAM") as dram_pool,
        ):
            # ---- constants ----
            w1_sb = const_pool.tile([D_IN, D_HID], f32r)
            nc.sync.dma_start(w1_sb[:], w1[:])
            w2_sb = const_pool.tile([D_HID, D_OUT], f32)
            nc.sync.dma_start(w2_sb[:], w2[:])

            # ---- XW1 = X @ W1, stored f32r as [128, KB*128] (kblk-major) ----
            xw1_all = const_pool.tile([128, N], f32r)
            with (
                tc.tile_pool(name="xT_pool", bufs=1) as xT_pool,
                tc.tile_pool(name="xw1_ps", bufs=2, space="PSUM") as xw1_ps_pool,
            ):
                xT_sb = xT_pool.tile([D_IN, N], f32r)
                nc.sync.dma_start(xT_sb[:], xT[:])
                for j in range(KB):
                    ps = xw1_ps_pool.tile([128, D_HID], f32, name=f"xw1ps{rep}_{j}",
                                          tag="xw1ps")
                    nc.tensor.matmul(ps[:], xT_sb[:, ts(j, 128)], w1_sb[:],
                                     start=True, stop=True)
                    nc.vector.tensor_copy(xw1_all[:, ts(j, 128)], ps[:])

            # ---- layer 1: psum_h[n, m] = sum_k XW1[k,n] * adjT[k,m] ----
            cached_slabs = []
            h1t = const_pool.tile([D_HID, SHARD], f32)
            with tc.tile_pool(name="l1_ps", bufs=1, space="PSUM") as l1_ps_pool:
                psum_h = l1_ps_pool.tile([D_HID, SHARD], f32)
                for j in range(KB):
                    if j < NCACHE:
                        slab = cache_pool.tile([128, SHARD], f32r,
                                               name=f"cslab{rep}_{j}",
                                               tag=f"cslab{j}")
                        cached_slabs.append(slab)
                    else:
                        slab = slab_pool.tile([128, SHARD], f32r,
                                              name=f"slab{rep}_{j}", tag="slab")
                    nc.sync.dma_start(slab[:], adjT[ts(j, 128), :])
                    for h in range(SHARD // 512):
                        nc.tensor.matmul(
                            psum_h[:, ts(h, 512)],
                            xw1_all[:, ts(j, 128)],
                            slab[:, ts(h, 512)],
                            start=(j == 0), stop=(j == KB - 1),
                        )
                nc.scalar.activation(h1t[:], psum_h[:], Relu)

            # ---- G_own = H1_own @ W2 (exact fp32), -> f32r DRAM, AllGather ----
            g_own_dram = dram_pool.tile([SHARD, D_OUT], f32r, name=f"g_own{rep}")
            g_all_dram = dram_pool.tile([N, D_OUT], f32r, addr_space="Shared",
                                        name=f"g_all{rep}")
            with tc.tile_pool(name="g_ps", bufs=2, space="PSUM") as g_ps_pool:
                for jl in range(KB_OWN):
                    ps = g_ps_pool.tile([128, D_OUT], f32, name=f"gps{rep}_{jl}",
                                        tag="gps")
                    nc.tensor.matmul(ps[:], h1t[:, ts(jl, 128)], w2_sb[:],
                                     start=True, stop=True)
                    g_sb = copy_pool.tile([128, D_OUT], f32r, name=f"gsb{rep}_{jl}",
                                          tag="gsb")
                    nc.vector.tensor_copy(g_sb[:], ps[:])
                    nc.sync.dma_start(g_own_dram[ts(jl, 128), :], g_sb[:])

            nc.gpsimd.collective_compute(
                "AllGather",
                mybir.AluOpType.bypass,
                replica_groups=[list(range(NCORES))],
                ins=[g_own_dram.opt()],
                outs=[g_all_dram.opt()],
            )

            # ---- layer 2: psum_o[c, m] = sum_k G[k,c] * adjT[k,m] ----
            with tc.tile_pool(name="l2_ps", bufs=1, space="PSUM") as l2_ps_pool:
                psum_o = [
                    l2_ps_pool.tile([128, SHARD], f32, name=f"l2ps{rep}_{ch}")
                    for ch in range(D_OUT // 128)
                ]
                for j in range(KB):
                    gt = gt_pool.tile([128, D_OUT], f32r, name=f"gt{rep}_{j}",
                                      tag="gt")
                    nc.sync.dma_start(gt[:], g_all_dram[ts(j, 128), :])
                    if j < NCACHE:
                        slab = cached_slabs[j]
                    else:
                        slab = slab_pool.tile([128, SHARD], f32r,
                                              name=f"slab2{rep}_{j}", tag="slab")
                        nc.sync.dma_start(slab[:], adjT[ts(j, 128), :])
                    for ch in range(D_OUT // 128):
                        for h in range(SHARD // 512):
                            nc.tensor.matmul(
                                psum_o[ch][:, ts(h, 512)],
                                gt[:, ts(ch, 128)],
                                slab[:, ts(h, 512)],
                                start=(j == 0), stop=(j == KB - 1),
                            )
                for ch in range(D_OUT // 128):
                    o_sb = copy_pool.tile([128, SHARD], f32, name=f"osb{rep}_{ch}",
                                          tag="osb", bufs=2)
                    nc.scalar.activation(o_sb[:], psum_o[ch][:], Relu)
                    nc.sync.dma_start(outT[ts(ch, 128), :], o_sb[:])

    with tile.TileContext(nc) as tc:
        for rep in range(reps):
            body(tc, rep)
    nc.compile()
    return nc


_NC_CACHE = {}


def get_nc(reps: int = 1):
    if reps not in _NC_CACHE:
        _NC_CACHE[reps] = _build_nc(reps)
    return _NC_CACHE[reps]


def make_in_maps(adj_matrix, node_embs, W1, W2):
    adj_matrix = np.ascontiguousarray(adj_matrix, dtype=np.float32)
    xT = np.ascontiguousarray(np.asarray(node_embs).T, dtype=np.float32)
    W1 = np.ascontiguousarray(W1, dtype=np.float32)
    W2 = np.ascontiguousarray(W2, dtype=np.float32)
    in_maps = []
    for i in range(NCORES):
        adjT_i = np.ascontiguousarray(adj_matrix[i * SHARD:(i + 1) * SHARD, :].T)
        in_maps.append({"adjT": adjT_i, "xT": xT, "w1": W1, "w2": W2})
    return in_maps


def kernel(adj_matrix, node_embs, W1, W2):
    import concourse.bass_utils as bass_utils

    nc = get_nc(reps=1)
    in_maps = make_in_maps(adj_matrix, node_embs, W1, W2)
    res = bass_utils.run_bass_kernel_spmd(nc, in_maps, core_ids=list(range(NCORES)))
    out = np.concatenate([r["outT"].T for r in res.results], axis=0)
    return np.ascontiguousarray(out, dtype=np.float32)


if __name__ == "__main__":
    rng = np.random.default_rng(0)
    adj = rng.random((N, N), dtype=np.float32)
    x = rng.standard_normal((N, D_IN)).astype(np.float32)
    W1 = (rng.standard_normal((D_IN, D_HID)) / np.sqrt(D_IN)).astype(np.float32)
    W2 = (rng.standard_normal((D_HID, D_OUT)) / np.sqrt(D_HID)).astype(np.float32)
    out = kernel(adj_matrix=adj, node_embs=x, W1=W1, W2=W2)
    h = np.maximum(adj @ (x @ W1), 0)
    expected = np.maximum(adj @ (h @ W2), 0)
    err = np.abs(out - expected).max() / np.abs(expected).max()
    print("rel err vs numpy:", err)


# revision 10
# speedup vs baseline: 98.8637x; 98.8637x over previous
"""Trainium2 Bass kernel for a 2-layer GCN (AttributeDecoder):

    out = relu(adj @ relu(adj @ (X @ W1)) @ W2)

with N=8192, D_IN=64, D_HID=128, D_OUT=256, all fp32.

Strategy (8 NeuronCores, SPMD):
  - Row-shard adj across cores: core i owns rows [1024*i, 1024*(i+1)).
    The host feeds each core adjT_i = adj[rows_i, :].T  ([8192, 1024],
    contiguous) so that on-chip tiles naturally have the contraction
    index k on the partition axis (the PE reduces over partitions).
  - X is passed transposed (xT [64, 8192]) and replicated; W1/W2 replicated.
  - On-chip per core:
      XW1 = X @ W1                     (f32r matmuls, [8192, 128] in SBUF)
      H1^T_own = relu(adj_i @ XW1)^T   psum[n,m] += XW1[kblk].T @ adjT slab
      PE-transpose H1^T_own -> H1_own ([1024, 128] k-major, f32r)
      AllGather(H1_own) -> H1 [8192, 128] (f32r, 0.5MB/rank, mesh regime)
      AH^T = (adj_i @ H1)^T            psum[n,m] += H1[kblk] @ adjT slab
      OUT^T_own = relu(W2^T @ AH^T)    four [128]x[128,512] matmuls + relu
    Layer 2 uses the associativity flip (adj@H1)@W2 instead of adj@(H1@W2):
    the streaming contraction is against H1's 128 columns, not G's 256 -
    half the PE work - and the G stage disappears entirely.
  - Host gathers outT_i ([256, 1024]) from each core and transposes back.

All adj/XW1/G matmuls use float32r (TF32-like, ~1.6e-4 rel err, full PE
rate at moving dim >= 256). Raw fp32 bits are DMA'd into f32r-declared
tensors directly; the hardware rounds internally - no cast pass needed.
The first NCACHE adjT slabs stay resident in SBUF after layer 1 so layer 2
skips their HBM reads (target_regime=memory).
"""

import numpy as np

N = 8192
D_IN, D_HID, D_OUT = 64, 128, 256
NCORES = 8
SHARD = N // NCORES  # 1024
KB = N // 128  # 64 k-blocks of 128
KB_OWN = SHARD // 128  # 8 k-blocks owned per core
# Number of k-block slabs ([128, SHARD] f32r = 512KB each) kept resident in
# SBUF after layer 1 so layer 2 skips their HBM reads.
NCACHE = 24


def _build_nc(reps: int = 1, ncache: int = None, slab_bufs: int = 4,
              gt_bufs: int = 3, l1_only: bool = False, no_coll: bool = False,
              gps_bufs: int = 2, gtc_bufs: int = 3):
    if ncache is None:
        ncache = NCACHE
    from concourse import bacc
    import concourse.mybir as mybir
    import concourse.tile as tile
    from concourse.bass import ts
    from concourse.masks import make_identity

    f32 = mybir.dt.float32
    f32r = mybir.dt.float32r
    Relu = mybir.ActivationFunctionType.Relu

    nc = bacc.Bacc("TRN2", target_bir_lowering=False, debug=False, num_devices=NCORES)

    adjT = nc.dram_tensor("adjT", [N, SHARD], f32r, kind="ExternalInput").ap()
    xT = nc.dram_tensor("xT", [D_IN, N], f32r, kind="ExternalInput").ap()
    w1 = nc.dram_tensor("w1", [D_IN, D_HID], f32r, kind="ExternalInput").ap()
    w2 = nc.dram_tensor("w2", [D_HID, D_OUT], f32r, kind="ExternalInput").ap()
    outT = nc.dram_tensor("outT", [D_OUT, SHARD], f32, kind="ExternalOutput").ap()

    def body(tc, rep):
        nc = tc.nc
        with (
            tc.tile_pool(name="const", bufs=1) as const_pool,
            tc.tile_pool(name="cache", bufs=1) as cache_pool,
            tc.tile_pool(name="slab", bufs=slab_bufs) as slab_pool,
            tc.tile_pool(name="gt", bufs=gt_bufs) as gt_pool,
            tc.tile_pool(name="copies", bufs=2) as copy_pool,
            tc.tile_pool(name="dram", bufs=1, space="DRAM") as dram_pool,
        ):
            # ---- constants ----
            w1_sb = const_pool.tile([D_IN, D_HID], f32r)
            nc.sync.dma_start(w1_sb[:], w1[:])
            w2_sb = const_pool.tile([D_HID, D_OUT], f32r)
            nc.sync.dma_start(w2_sb[:], w2[:])

            # ---- XW1 = X @ W1, stored f32r as [128, KB*128] (kblk-major) ----
            xw1_all = const_pool.tile([128, N], f32r)
            with (
                tc.tile_pool(name="xT_pool", bufs=1) as xT_pool,
                tc.tile_pool(name="xw1_ps", bufs=2, space="PSUM") as xw1_ps_pool,
            ):
                xT_sb = xT_pool.tile([D_IN, N], f32r)
                nc.sync.dma_start(xT_sb[:], xT[:])
                for j in range(KB):
                    ps = xw1_ps_pool.tile([128, D_HID], f32, name=f"xw1ps{rep}_{j}",
                                          tag="xw1ps")
                    nc.tensor.matmul(ps[:], xT_sb[:, ts(j, 128)], w1_sb[:],
                                     start=True, stop=True)
                    nc.vector.tensor_copy(xw1_all[:, ts(j, 128)], ps[:])

            # ---- layer 1: psum_h[n, m] = sum_k XW1[k,n] * adjT[k,m] ----
            cached_slabs = []
            h1r = const_pool.tile([D_HID, SHARD], f32r)
            with tc.tile_pool(name="l1_ps", bufs=1, space="PSUM") as l1_ps_pool:
                psum_h = l1_ps_pool.tile([D_HID, SHARD], f32)
                for j in range(KB):
                    if j < ncache:
                        slab = cache_pool.tile([128, SHARD], f32r,
                                               name=f"cslab{rep}_{j}",
                                               tag=f"cslab{j}")
                        cached_slabs.append(slab)
                    else:
                        slab = slab_pool.tile([128, SHARD], f32r,
                                              name=f"slab{rep}_{j}", tag="slab")
                    nc.sync.dma_start(slab[:], adjT[ts(j, 128), :])
                    for h in range(SHARD // 512):
                        nc.tensor.matmul(
                            psum_h[:, ts(h, 512)],
                            xw1_all[:, ts(j, 128)],
                            slab[:, ts(h, 512)],
                            start=(j == 0), stop=(j == KB - 1),
                        )
                # relu + round to f32r in one pass (H1T_own, to be gathered)
                nc.scalar.activation(h1r[:], psum_h[:], Relu)

            if l1_only:
                nc.sync.dma_start(outT[ts(0, 128), :], h1r[:].bitcast(f32))
                return

            # ---- PE-transpose H1T_own -> 8 pieces [128, D_HID] k-major ----
            # Each piece is AllGather'd separately (64KB/rank, pipelined) so
            # layer 2 starts as soon as the first piece lands; the remaining
            # collectives stream behind layer-2 compute.
            ident_f32 = const_pool.tile([128, 128], f32, name=f"identf{rep}")
            make_identity(nc, ident_f32[:])
            identity = const_pool.tile([128, 128], f32r, name=f"ident{rep}")
            nc.vector.tensor_copy(identity[:], ident_f32[:])
            h1_own_pieces = []
            h1_all_pieces = []
            with tc.tile_pool(name="tr_ps", bufs=2, space="PSUM") as tr_ps_pool:
                for jl in range(KB_OWN):
                    tps = tr_ps_pool.tile([128, D_HID], f32r,
                                          name=f"tps{rep}_{jl}", tag="tps")
                    nc.tensor.transpose(tps[:], h1r[:, ts(jl, 128)], identity[:])
                    tsb = copy_pool.tile([128, D_HID], f32r,
                                         name=f"tsb{rep}_{jl}", tag="tsb", bufs=3)
                    nc.vector.tensor_copy(tsb[:], tps[:])
                    own = dram_pool.tile([128, D_HID], f32r,
                                         name=f"h1own{rep}_{jl}")
                    nc.sync.dma_start(own[:], tsb[:])
                    h1_own_pieces.append(own)
                    h1_all_pieces.append(dram_pool.tile(
                        [NCORES * 128, D_HID], f32r, addr_space="Shared",
                        name=f"h1all{rep}_{jl}"))

            if not no_coll:
                for jl in range(KB_OWN):
                    nc.gpsimd.collective_compute(
                        "AllGather",
                        mybir.AluOpType.bypass,
                        replica_groups=[list(range(NCORES))],
                        ins=[h1_own_pieces[jl].opt()],
                        outs=[h1_all_pieces[jl].opt()],
                    )

            # ---- layer 2: psum_ah[n, m] += H1[kblk j][k,n] * adjT[k,m] ----
            # Piece p's gathered output holds global k-blocks j = r*KB_OWN + p
            # (rank r's block at rows [128r, 128(r+1))). Iterate p-major so
            # work unlocks in collective completion order; the contraction is
            # order-invariant.
            with tc.tile_pool(name="l2_ps", bufs=1, space="PSUM") as l2_ps_pool:
                psum_ah = l2_ps_pool.tile([D_HID, SHARD], f32, name=f"l2ps{rep}")
                for p in range(KB_OWN):
                    for r in range(NCORES):
                        j = r * KB_OWN + p
                        h1kn = gt_pool.tile([128, D_HID], f32r,
                                            name=f"h1kn{rep}_{j}", tag="h1kn")
                        if no_coll:
                            nc.sync.dma_start(h1kn[:], h1_own_pieces[p][:])
                        else:
                            nc.sync.dma_start(
                                h1kn[:], h1_all_pieces[p][ts(r, 128), :])
                        if j < ncache:
                            slab = cached_slabs[j]
                        else:
                            slab = slab_pool.tile([128, SHARD], f32r,
                                                  name=f"slab2{rep}_{j}", tag="slab")
                            nc.sync.dma_start(slab[:], adjT[ts(j, 128), :])
                        for h in range(SHARD // 512):
                            nc.tensor.matmul(
                                psum_ah[:, ts(h, 512)],
                                h1kn[:],
                                slab[:, ts(h, 512)],
                                start=(p == 0 and r == 0),
                                stop=(p == KB_OWN - 1 and r == NCORES - 1),
                            )
                # round AH^T to f32r
                ah_sb = copy_pool.tile([D_HID, SHARD], f32r, name=f"ahsb{rep}",
                                       tag="ahsb", bufs=1)
                nc.vector.tensor_copy(ah_sb[:], psum_ah[:])

            # ---- OUT^T = relu(W2^T @ AH^T): psum_of[c,m] = sum_n W2[n,c]*AH^T[n,m]
            with tc.tile_pool(name="of_ps", bufs=1, space="PSUM") as of_ps_pool:
                for ch in range(D_OUT // 128):
                    psum_of = of_ps_pool.tile([128, SHARD], f32,
                                              name=f"ofps{rep}_{ch}", tag="ofps",
                                              bufs=2)
                    for h in range(SHARD // 512):
                        nc.tensor.matmul(
                            psum_of[:, ts(h, 512)],
                            w2_sb[:, ts(ch, 128)],
                            ah_sb[:, ts(h, 512)],
                            start=True, stop=True,
                        )
                    o_sb = copy_pool.tile([128, SHARD], f32, name=f"osb{rep}_{ch}",
                                          tag="osb", bufs=2)
                    nc.scalar.activation(o_sb[:], psum_of[:], Relu)
                    nc.sync.dma_start(outT[ts(ch, 128), :], o_sb[:])

    with tile.TileContext(nc) as tc:
        for rep in range(reps):
            body(tc, rep)
    nc.compile()
    return nc


_NC_CACHE = {}


def get_nc(reps: int = 1, **opts):
    key = (reps, tuple(sorted(opts.items())))
    if key not in _NC_CACHE:
        _NC_CACHE[key] = _build_nc(reps, **opts)
    return _NC_CACHE[key]


def make_in_maps(adj_matrix, node_embs, W1, W2):
    adj_matrix = np.ascontiguousarray(adj_matrix, dtype=np.float32)
    xT = np.ascontiguousarray(np.asarray(node_embs).T, dtype=np.float32)
    W1 = np.ascontiguousarray(W1, dtype=np.float32)
    W2 = np.ascontiguousarray(W2, dtype=np.float32)
    in_maps = []
    for i in range(NCORES):
        adjT_i = np.ascontiguousarray(adj_matrix[i * SHARD:(i + 1) * SHARD, :].T)
        in_maps.append({"adjT": adjT_i, "xT": xT, "w1": W1, "w2": W2})
    return in_maps


def kernel(adj_matrix, node_embs, W1, W2):
    import concourse.bass_utils as bass_utils

    nc = get_nc(reps=1)
    in_maps = make_in_maps(adj_matrix, node_embs, W1, W2)
    res = bass_utils.run_bass_kernel_spmd(nc, in_maps, core_ids=list(range(NCORES)))
    out = np.concatenate([r["outT"].T for r in res.results], axis=0)
    return np.ascontiguousarray(out, dtype=np.float32)


if __name__ == "__main__":
    rng = np.random.default_rng(0)
    adj = rng.random((N, N), dtype=np.float32)
    x = rng.standard_normal((N, D_IN)).astype(np.float32)
    W1 = (rng.standard_normal((D_IN, D_HID)) / np.sqrt(D_IN)).astype(np.float32)
    W2 = (rng.standard_normal((D_HID, D_OUT)) / np.sqrt(D_HID)).astype(np.float32)
    out = kernel(adj_matrix=adj, node_embs=x, W1=W1, W2=W2)
    h = np.maximum(adj @ (x @ W1), 0)
    expected = np.maximum(adj @ (h @ W2), 0)
    err = np.abs(out - expected).max() / np.abs(expected).max()
    print("rel err vs numpy:", err)
